# revision 40
# baseline (speedup 1.0000x reference)
"""nn_BERT_FOL_T — BERT-base forward + label-logit head on 8 TRN2 NeuronCores.

Sharding: data-parallel over batch (B=32 -> 4 seqs/core), BERT weights
replicated per core (streamed HBM->SBUF as bf16). The 12 transformer layers
run on-device; embedding gather + masked-mean pooling + dense2 + label-logit
matmul are host-side (0.03% of FLOPs).

Device kernel layout: activations feature-major X^T [6][128 h, 512 tok],
fp32 residual, bf16 matmul operands. All matmuls contract over the SBUF
partition dim; LayerNorm reduces over partitions via ones-vector matmuls and
broadcasts per-token stats back with a ones-row outer-product matmul.
"""
import os
import sys
import types

sys.path.insert(0, "/opt/trn_rl_repo")
os.environ.setdefault("BASS_NEVER_TRACE", "1")

import numpy as np
import ml_dtypes
from contextlib import ExitStack

import concourse.bass as bass
import concourse.tile as tile
from concourse import mybir
from concourse.masks import make_identity
from concourse.tile import ScopedClock

# ---------------------------------------------------------------------------
# Workarounds for this walrus build (max ONE sync wait per instruction).
# ---------------------------------------------------------------------------
_MAX_WAITS = 1


def _patched_drain_and_barrier(self, tick_clock, wait_clock):
    nc = self.nc
    probe = nc.sync.nop(nofuse=True)
    wait_clock.add_sem_waits(probe.ins, ScopedClock({None: tick_clock.global_clock}))
    si = probe.ins.sync_info
    waits = list(si.on_wait or []) if si is not None else []
    if len(waits) > _MAX_WAITS:
        si.on_wait = waits[:_MAX_WAITS]
        rest = waits[_MAX_WAITS:]
        while rest:
            chunk, rest = rest[:_MAX_WAITS], rest[_MAX_WAITS:]
            nop = nc.sync.nop(nofuse=True)
            nsi = nop.ins.sync_info
            if nsi is None:
                nop.ins.sync_info = mybir.SyncInfo(on_wait=chunk, on_update=[])
            else:
                nsi.on_wait = chunk
    nc.sync.drain()
    nc.all_engine_barrier()
    assert self.sems is not None
    popped = nc._tile_sem_poison_stack.pop()
    assert popped is self._sem_poison
    nc.clear_and_free_semaphores(list(self.sems.allocated().values()))
    nc.all_engine_barrier()


def _split_waits_in_ordered(ordered):
    for bb_name, insts in ordered.items():
        new_list = []
        for inst in insts:
            si = getattr(inst, "sync_info", None)
            waits = list(si.on_wait) if si is not None and si.on_wait else []
            if len(waits) > _MAX_WAITS and type(inst).__name__.startswith("Inst"):
                keep = waits[-_MAX_WAITS:]
                hoist = waits[:-_MAX_WAITS]
                for k, cs in enumerate(range(0, len(hoist), _MAX_WAITS)):
                    chunk = hoist[cs:cs + _MAX_WAITS]
                    nop = mybir.InstNoOp(
                        name=f"{inst.name}-wsplit{k}",
                        engine=inst.engine,
                        bass_nofuse=True,
                        sync_info=mybir.SyncInfo(on_wait=chunk, on_update=[]),
                    )
                    new_list.append(nop)
                si.on_wait = keep
            new_list.append(inst)
        ordered[bb_name] = new_list
    return ordered


_orig_lower = tile.TileContext._lower_ordered_insts


def _patched_lower_ordered_insts(self, ordered):
    return _orig_lower(self, _split_waits_in_ordered(ordered))


def _install_patches():
    tile.TileContext._drain_and_barrier = _patched_drain_and_barrier
    tile.TileContext._lower_ordered_insts = _patched_lower_ordered_insts


def _install_ntff_hook():
    """The image's antenv lacks axon_hooks; synthesize it so trace=True works."""
    if "antenv.axon_hooks" in sys.modules:
        return
    mod = types.ModuleType("antenv.axon_hooks")
    _hook = [None]
    mod.set_axon_ntff_profile_hook = lambda h: _hook.__setitem__(0, h)
    mod.get_axon_ntff_profile_hook = lambda: _hook[0]
    sys.modules["antenv.axon_hooks"] = mod
    try:
        from trn_agent_boot.trn_boot import _ntff_profile_via_ctypes
        _hook[0] = _ntff_profile_via_ctypes("/opt/axon/libaxon_pjrt.so")
    except Exception:
        pass


_install_patches()
_install_ntff_hook()

# ---------------------------------------------------------------------------
# Device kernel
# ---------------------------------------------------------------------------
F32 = mybir.dt.float32
BF16 = mybir.dt.bfloat16
OP = mybir.AluOpType
AF = mybir.ActivationFunctionType

B, S, H, NH, D, FF, V = 32, 128, 768, 12, 64, 3072, 30522
N_CORES = 8
B_LOC = B // N_CORES          # 4 sequences per core
T = B_LOC * S                 # 512 tokens per core
H_TILES = 6
FF_TILES = 24
EPS = 1e-12
INV_SQRT_D = 0.125
INV_H = 1.0 / 768.0


def build_bert(n_layers=12, use_bias=False, use_ln_affine=False):
    nc = bass.Bass()
    x0t = nc.dram_tensor("x0t", [H_TILES, 128, T], F32, kind="ExternalInput")
    wblk = nc.dram_tensor("wblk", [n_layers * 12, 128, 4608], BF16,
                          kind="ExternalInput")
    if use_bias:
        pbias = nc.dram_tensor("pbias", [n_layers, 128, 48], F32,
                               kind="ExternalInput")
        bvrow = nc.dram_tensor("bvrow", [n_layers, 1, 768], F32,
                               kind="ExternalInput")
    if use_ln_affine:
        lnsb = nc.dram_tensor("lnsb", [n_layers, 128, 24], F32,
                              kind="ExternalInput")
    xout = nc.dram_tensor("xout", [H_TILES, 128, T], F32, kind="ExternalOutput")

    with tile.TileContext(nc) as tc, ExitStack() as ctx:
        act = ctx.enter_context(tc.tile_pool(name="act", bufs=1))
        wp = ctx.enter_context(tc.tile_pool(name="wp", bufs=10))
        tmp = ctx.enter_context(tc.tile_pool(name="tmp", bufs=3))
        st = ctx.enter_context(tc.tile_pool(name="st", bufs=6))
        attp = ctx.enter_context(tc.tile_pool(name="attp", bufs=3))
        mm = ctx.enter_context(tc.tile_pool(name="mm", bufs=3, space="PSUM"))
        sc = ctx.enter_context(tc.tile_pool(name="sc", bufs=3, space="PSUM"))
        tp = ctx.enter_context(tc.tile_pool(name="tp", bufs=2, space="PSUM"))

        X = [act.tile([128, T], F32, tag=f"X{i}", name=f"X{i}")
             for i in range(H_TILES)]
        Xn = [act.tile([128, T], BF16, tag=f"Xn{i}", name=f"Xn{i}")
              for i in range(H_TILES)]
        QT = [act.tile([128, T], BF16, tag=f"QT{i}", name=f"QT{i}")
              for i in range(H_TILES)]
        KT = [act.tile([128, T], BF16, tag=f"KT{i}", name=f"KT{i}")
              for i in range(H_TILES)]
        Vt = [act.tile([128, 768], BF16, tag=f"V{i}", name=f"V{i}")
              for i in range(B_LOC)]
        CT = [act.tile([128, T], BF16, tag=f"CT{i}", name=f"CT{i}")
              for i in range(H_TILES)]
        G = [act.tile([128, T], BF16, tag=f"G{i}", name=f"G{i}")
             for i in range(FF_TILES)]
        Xc = [act.tile([128, T], BF16, tag=f"Xc{i}", name=f"Xc{i}")
              for i in range(H_TILES)]
        X2 = [act.tile([128, T], BF16, tag=f"X2{i}", name=f"X2{i}")
              for i in range(H_TILES)]
        Ab = act.tile([128, T], F32, tag="Ab", name="Ab")
        Bb = act.tile([128, T], F32, tag="Bb", name="Bb")
        rstd = act.tile([128, T], F32, tag="rstd", name="rstd")
        cc = act.tile([128, T], F32, tag="cc", name="cc")
        ident = act.tile([128, 128], BF16, tag="ident", name="ident")
        ones_col = act.tile([128, 1], BF16, tag="ones_col", name="ones_col")
        ones_row = act.tile([1, 128], BF16, tag="ones_row", name="ones_row")
        eps_col = act.tile([128, 1], F32, tag="eps_col", name="eps_col")
        Sbf = act.tile([1, T], BF16, tag="Sbf", name="Sbf")
        S2bf = act.tile([1, T], BF16, tag="S2bf", name="S2bf")

        make_identity(nc, ident[:])
        nc.vector.memset(ones_col[:], 1.0)
        nc.vector.memset(ones_row[:], 1.0)
        nc.vector.memset(eps_col[:], EPS)

        for i in range(H_TILES):
            nc.sync.dma_start(X[i][:], x0t[i])
            nc.vector.tensor_copy(Xn[i][:], X[i][:])

        def layernorm(s_ap=None, b_ap=None, final_out=None):
            """X (f32 residual) -> LN in place -> Xn (bf16 cast).
            If final_out is set, DMA the normalized f32 out instead."""
            for i in range(H_TILES):
                nc.vector.tensor_copy(Xc[i][:], X[i][:])
                nc.vector.tensor_tensor(out=X2[i][:], in0=Xc[i][:],
                                        in1=Xc[i][:], op=OP.mult)
            s_ps = mm.tile([1, T], F32, tag="mm", name="s_ps")
            for i in range(H_TILES):
                nc.tensor.matmul(s_ps[:], ones_col[:], Xc[i][:],
                                 start=(i == 0), stop=(i == H_TILES - 1))
            nc.vector.tensor_copy(Sbf[:], s_ps[:])
            s2_ps = mm.tile([1, T], F32, tag="mm", name="s2_ps")
            for i in range(H_TILES):
                nc.tensor.matmul(s2_ps[:], ones_col[:], X2[i][:],
                                 start=(i == 0), stop=(i == H_TILES - 1))
            nc.vector.tensor_copy(S2bf[:], s2_ps[:])
            a_ps = mm.tile([128, T], F32, tag="mm", name="a_ps")
            nc.tensor.matmul(a_ps[:], ones_row[:], Sbf[:], start=True, stop=True)
            nc.vector.tensor_copy(Ab[:], a_ps[:])
            b_ps = mm.tile([128, T], F32, tag="mm", name="b_ps")
            nc.tensor.matmul(b_ps[:], ones_row[:], S2bf[:], start=True, stop=True)
            nc.vector.tensor_copy(Bb[:], b_ps[:])
            t1 = tmp.tile([128, T], F32, tag="tmp", name="t1")
            nc.vector.tensor_tensor(out=t1[:], in0=Ab[:], in1=Ab[:], op=OP.mult)
            t2 = tmp.tile([128, T], F32, tag="tmp", name="t2")
            nc.vector.scalar_tensor_tensor(out=t2[:], in0=t1[:], scalar=-INV_H,
                                           in1=Bb[:], op0=OP.mult, op1=OP.add)
            sd = tmp.tile([128, T], F32, tag="tmp", name="sd")
            nc.scalar.activation(sd[:], t2[:], AF.Sqrt, bias=eps_col[:],
                                 scale=INV_H)
            nc.vector.reciprocal(rstd[:], sd[:])
            nc.vector.scalar_tensor_tensor(out=cc[:], in0=Ab[:], scalar=INV_H,
                                           in1=rstd[:], op0=OP.mult, op1=OP.mult)
            for i in range(H_TILES):
                u = tmp.tile([128, T], F32, tag="tmp", name="u")
                nc.vector.tensor_tensor(out=u[:], in0=X[i][:], in1=rstd[:],
                                        op=OP.mult)
                if final_out is None:
                    if s_ap is not None:
                        u2 = tmp.tile([128, T], F32, tag="tmp", name="u2")
                        nc.vector.tensor_tensor(out=u2[:], in0=u[:], in1=cc[:],
                                                op=OP.subtract)
                        nc.scalar.activation(X[i][:], u2[:], AF.Identity,
                                             bias=b_ap(i), scale=s_ap(i))
                    else:
                        nc.vector.tensor_tensor(out=X[i][:], in0=u[:], in1=cc[:],
                                                op=OP.subtract)
                    nc.vector.tensor_copy(Xn[i][:], X[i][:])
                else:
                    xo = tmp.tile([128, T], F32, tag="xo", name="xo")
                    if s_ap is not None:
                        u2 = tmp.tile([128, T], F32, tag="tmp", name="u2f")
                        nc.vector.tensor_tensor(out=u2[:], in0=u[:], in1=cc[:],
                                                op=OP.subtract)
                        nc.scalar.activation(xo[:], u2[:], AF.Identity,
                                             bias=b_ap(i), scale=s_ap(i))
                    else:
                        nc.vector.tensor_tensor(out=xo[:], in0=u[:], in1=cc[:],
                                                op=OP.subtract)
                    nc.sync.dma_start(final_out[i], xo[:])

        for layer in range(n_layers):
            base = layer * 12
            wq = wp.tile([128, 4608], BF16, tag="w", name="wq")
            nc.sync.dma_start(wq[:], wblk[base + 0])
            wk = wp.tile([128, 4608], BF16, tag="w", name="wk")
            nc.sync.dma_start(wk[:], wblk[base + 1])
            wv = wp.tile([128, 4608], BF16, tag="w", name="wv")
            nc.sync.dma_start(wv[:], wblk[base + 2])
            wo = wp.tile([128, 4608], BF16, tag="w", name="wo")
            nc.sync.dma_start(wo[:], wblk[base + 3])

            if use_bias:
                pb = st.tile([128, 48], F32, tag="pb", name="pb")
                nc.sync.dma_start(pb[:], pbias[layer])
                bvb = st.tile([128, 768], F32, tag="bvb", name="bvb")
                nc.sync.dma_start(bvb[:], bvrow[layer].to_broadcast([128, 768]))
            if use_ln_affine:
                lnt = st.tile([128, 24], F32, tag="lnt", name="lnt")
                nc.sync.dma_start(lnt[:], lnsb[layer])

            # ---- Q/K projections (feature-major) ----
            for (wsb, out_tiles, bcol) in ((wq, QT, 0), (wk, KT, 6)):
                for mt in range(H_TILES):
                    ps = mm.tile([128, T], F32, tag="mm", name="ps")
                    for kt in range(H_TILES):
                        nc.tensor.matmul(
                            ps[:],
                            wsb[:, kt * 768 + mt * 128:kt * 768 + mt * 128 + 128],
                            Xn[kt][:], start=(kt == 0), stop=(kt == H_TILES - 1))
                    if use_bias:
                        nc.scalar.activation(out_tiles[mt][:], ps[:], AF.Identity,
                                             bias=pb[:, bcol + mt:bcol + mt + 1])
                    else:
                        nc.vector.tensor_copy(out_tiles[mt][:], ps[:])
            # ---- V projection (token-major) ----
            for mt in range(B_LOC):
                for half in range(2):
                    ps = mm.tile([128, T], F32, tag="mm", name="vps")
                    for kt in range(H_TILES):
                        nc.tensor.matmul(
                            ps[:, 0:384],
                            Xn[kt][:, mt * 128:(mt + 1) * 128],
                            wv[:, kt * 768 + half * 384:kt * 768 + half * 384 + 384],
                            start=(kt == 0), stop=(kt == H_TILES - 1))
                    dst = Vt[mt][:, half * 384:(half + 1) * 384]
                    if use_bias:
                        nc.vector.scalar_tensor_tensor(
                            out=dst, in0=ps[:, 0:384], scalar=1.0,
                            in1=bvb[:, half * 384:(half + 1) * 384],
                            op0=OP.mult, op1=OP.add)
                    else:
                        nc.vector.tensor_copy(dst, ps[:, 0:384])

            # ---- attention (4 batches x 6 head-pairs) ----
            for b in range(B_LOC):
                tsl = slice(b * 128, (b + 1) * 128)
                for ht in range(H_TILES):
                    attTs = []
                    for sub in range(2):
                        h = ht * 2 + sub
                        hp = sub * 64
                        s_ps = sc.tile([128, 128], F32, tag="sc", name="s_ps")
                        nc.tensor.matmul(s_ps[:], QT[ht][hp:hp + 64, tsl],
                                         KT[ht][hp:hp + 64, tsl],
                                         start=True, stop=True)
                        esum = st.tile([128, 1], F32, tag="es", name="esum")
                        att = attp.tile([128, 128], BF16, tag="att", name="att")
                        nc.scalar.activation(att[:], s_ps[:], AF.Exp,
                                             scale=INV_SQRT_D, accum_out=esum[:])
                        rec = st.tile([128, 1], F32, tag="rec", name="rec")
                        nc.vector.reciprocal(rec[:], esum[:])
                        attn = attp.tile([128, 128], BF16, tag="attn", name="attn")
                        nc.vector.tensor_scalar_mul(attn[:], att[:], rec[:])
                        t_ps = tp.tile([128, 128], BF16, tag="tp", name="t_ps")
                        nc.tensor.transpose(t_ps[:], attn[:], ident[:])
                        attT = attp.tile([128, 128], BF16, tag="attT", name="attT")
                        nc.vector.tensor_copy(attT[:], t_ps[:])
                        attTs.append(attT)
                    c_ps = sc.tile([128, 128], F32, tag="sc", name="c_ps")
                    nc.tensor.matmul(c_ps[0:64, :],
                                     Vt[b][:, ht * 128:ht * 128 + 64],
                                     attTs[0][:], start=True, stop=True,
                                     tile_position=(0, 0))
                    nc.tensor.matmul(c_ps[64:128, :],
                                     Vt[b][:, ht * 128 + 64:ht * 128 + 128],
                                     attTs[1][:], start=True, stop=True,
                                     tile_position=(0, 64))
                    nc.vector.tensor_copy(CT[ht][:, tsl], c_ps[:])

            # ---- O projection + residual ----
            for mt in range(H_TILES):
                ps = mm.tile([128, T], F32, tag="mm", name="ops")
                for kt in range(H_TILES):
                    nc.tensor.matmul(
                        ps[:], wo[:, kt * 768 + mt * 128:kt * 768 + mt * 128 + 128],
                        CT[kt][:], start=(kt == 0), stop=(kt == H_TILES - 1))
                bo_s = pb[:, 12 + mt:13 + mt] if use_bias else 0.0
                nc.vector.scalar_tensor_tensor(out=X[mt][:], in0=ps[:],
                                               scalar=bo_s, in1=X[mt][:],
                                               op0=OP.add, op1=OP.add)

            layernorm(
                s_ap=(lambda i: lnt[:, 0 + i:1 + i]) if use_ln_affine else None,
                b_ap=(lambda i: lnt[:, 6 + i:7 + i]) if use_ln_affine else None)

            # ---- FFN ----
            w1c = []
            for c in range(4):
                wt_ = wp.tile([128, 4608], BF16, tag="w", name=f"w1c{c}")
                nc.sync.dma_start(wt_[:], wblk[base + 4 + c])
                w1c.append(wt_)
            for fc in range(4):
                for fm in range(H_TILES):
                    g = fc * 6 + fm
                    ps = mm.tile([128, T], F32, tag="mm", name="gps")
                    for kt in range(H_TILES):
                        nc.tensor.matmul(
                            ps[:],
                            w1c[fc][:, kt * 768 + fm * 128:kt * 768 + fm * 128 + 128],
                            Xn[kt][:], start=(kt == 0), stop=(kt == H_TILES - 1))
                    b1_s = pb[:, 24 + g:25 + g] if use_bias else 0.0
                    nc.scalar.activation(G[g][:], ps[:], AF.Gelu_apprx_tanh,
                                         bias=b1_s if use_bias else 0.0)
            w2c = []
            for c in range(4):
                wt_ = wp.tile([128, 4608], BF16, tag="w", name=f"w2c{c}")
                nc.sync.dma_start(wt_[:], wblk[base + 8 + c])
                w2c.append(wt_)
            for mt in range(H_TILES):
                ps = mm.tile([128, T], F32, tag="mm", name="yps")
                for kc in range(4):
                    for kk in range(H_TILES):
                        nc.tensor.matmul(
                            ps[:],
                            w2c[kc][:, kk * 768 + mt * 128:kk * 768 + mt * 128 + 128],
                            G[kc * 6 + kk][:],
                            start=(kc == 0 and kk == 0),
                            stop=(kc == 3 and kk == H_TILES - 1))
                b2_s = pb[:, 18 + mt:19 + mt] if use_bias else 0.0
                nc.vector.scalar_tensor_tensor(out=X[mt][:], in0=ps[:],
                                               scalar=b2_s, in1=X[mt][:],
                                               op0=OP.add, op1=OP.add)

            is_last = layer == n_layers - 1
            layernorm(
                s_ap=(lambda i: lnt[:, 12 + i:13 + i]) if use_ln_affine else None,
                b_ap=(lambda i: lnt[:, 18 + i:19 + i]) if use_ln_affine else None,
                final_out=xout if is_last else None)
    return nc


# ---------------------------------------------------------------------------
# v2 device kernel: no PE transposes, row-stat LN, half-token wavefront.
# ---------------------------------------------------------------------------
_V2_SKIP_ATTN = False
_V2_SKIP_LN = False
_V2_LN_LEVEL = 5  # 2=+S2 colsum, 3=+row stats, 4=+bcast MMs, 5=full finish


def build_bert_v2(n_layers=12):
    """Fast path for the no-bias / no-LN-affine parameterization.

    Layout: activations feature-major X^T as 6 tiles [128 h, 512 tok];
    every GEMM phase runs per token-half h (256 tok) so LayerNorm vector
    work for one half overlaps PE work on the other half.  Attention uses
    [k,q]-oriented scores (no transposes): ctx_u = V^T @ exp(K^T Q), row
    sums via a ones-matmul, 1/sum via reciprocal_approx_fast, and the
    per-column normalizer broadcast to 128 partitions with an SBUF DMA.
    """
    nc = bass.Bass()
    x0t = nc.dram_tensor("x0t", [H_TILES, 128, T], F32, kind="ExternalInput")
    wblk = nc.dram_tensor("wblk", [n_layers * 12, 128, 4608], BF16,
                          kind="ExternalInput")
    xout = nc.dram_tensor("xout", [H_TILES, 128, T], F32, kind="ExternalOutput")

    HS = T // 2                   # 256 tokens per half

    with tile.TileContext(nc) as tc, ExitStack() as ctx:
        act = ctx.enter_context(tc.tile_pool(name="act", bufs=1))
        wp = ctx.enter_context(tc.tile_pool(name="wp", bufs=10))
        tmp = ctx.enter_context(tc.tile_pool(name="tmp", bufs=3))
        sxp = ctx.enter_context(tc.tile_pool(name="sxp", bufs=3))
        rwp = ctx.enter_context(tc.tile_pool(name="rwp", bufs=4))
        mm = ctx.enter_context(tc.tile_pool(name="mm", bufs=2, space="PSUM"))
        sc = ctx.enter_context(tc.tile_pool(name="sc", bufs=2, space="PSUM"))
        cp = ctx.enter_context(tc.tile_pool(name="cp", bufs=2, space="PSUM"))
        rws = ctx.enter_context(tc.tile_pool(name="rws", bufs=2, space="PSUM"))

        X = [act.tile([128, T], F32, tag=f"X{i}", name=f"X{i}")
             for i in range(H_TILES)]
        Xn = [act.tile([128, T], BF16, tag=f"Xn{i}", name=f"Xn{i}")
              for i in range(H_TILES)]
        QT = [act.tile([128, T], BF16, tag=f"QT{i}", name=f"QT{i}")
              for i in range(H_TILES)]
        KT = [act.tile([128, T], BF16, tag=f"KT{i}", name=f"KT{i}")
              for i in range(H_TILES)]
        Vt = [act.tile([128, 768], BF16, tag=f"V{i}", name=f"V{i}")
              for i in range(B_LOC)]
        CT = [act.tile([128, T], BF16, tag=f"CT{i}", name=f"CT{i}")
              for i in range(H_TILES)]
        G = [act.tile([128, T], BF16, tag=f"G{i}", name=f"G{i}")
             for i in range(FF_TILES)]
        X2b = [act.tile([128, HS], BF16, tag=f"X2b{i}", name=f"X2b{i}")
               for i in range(H_TILES)]
        ones_cf = act.tile([128, 1], F32, tag="ones_cf", name="ones_cf")
        ones_cb = act.tile([128, 1], BF16, tag="ones_cb", name="ones_cb")
        ones_rf = act.tile([1, 128], F32, tag="ones_rf", name="ones_rf")
        ones_rb = act.tile([1, 128], BF16, tag="ones_rb", name="ones_rb")
        eps_c = act.tile([128, 1], F32, tag="eps_c", name="eps_c")
        ident = act.tile([128, 128], BF16, tag="ident", name="ident")

        nc.vector.memset(ones_cf[:], 1.0)
        nc.vector.memset(ones_cb[:], 1.0)
        nc.vector.memset(ones_rf[:], 1.0)
        nc.vector.memset(ones_rb[:], 1.0)
        nc.vector.memset(eps_c[:], EPS)
        make_identity(nc, ident[:])

        for i in range(H_TILES):
            nc.sync.dma_start(X[i][:], x0t[i])
            nc.vector.tensor_copy(Xn[i][:], X[i][:])

        def qk_chain_one(wsb, out_tiles, h, mt):
            hsl = slice(h * HS, (h + 1) * HS)
            ps = mm.tile([128, HS], F32, tag="mm", name="ps")
            for kt in range(H_TILES):
                nc.tensor.matmul(
                    ps[:],
                    wsb[:, kt * 768 + mt * 128:kt * 768 + mt * 128 + 128],
                    Xn[kt][:, hsl], start=(kt == 0),
                    stop=(kt == H_TILES - 1))
            nc.vector.tensor_copy(out_tiles[mt][:, hsl], ps[:])

        def qk_chain(wsb, out_tiles, h):
            for mt in range(H_TILES):
                qk_chain_one(wsb, out_tiles, h, mt)

        def v_chains(wv, b):
            # two interleaved N=384 chains so LDWEIGHTS (shared stationary
            # Xn slice) hides under two matmuls
            tsl = slice(b * 128, (b + 1) * 128)
            psA = mm.tile([128, 384], F32, tag="mm", name="vpsA")
            psB = mm.tile([128, 384], F32, tag="mm", name="vpsB")
            for kt in range(H_TILES):
                nc.tensor.matmul(psA[:], Xn[kt][:, tsl],
                                 wv[:, kt * 768:kt * 768 + 384],
                                 start=(kt == 0), stop=(kt == H_TILES - 1))
                nc.tensor.matmul(psB[:], Xn[kt][:, tsl],
                                 wv[:, kt * 768 + 384:kt * 768 + 768],
                                 start=(kt == 0), stop=(kt == H_TILES - 1))
            nc.vector.tensor_copy(Vt[b][:, 0:384], psA[:])
            nc.vector.tensor_copy(Vt[b][:, 384:768], psB[:])

        # ---- attention units, software-pipelined ----
        # [q,k] scores; exp+accum rowsums; [128,1] recips; transpose the
        # normalized weights via PE into spare columns of the score bank.
        def attn_units(units, fillers=()):
            fillers = list(fillers)
            n = len(units)
            st_score = [None] * n
            st_attq = [None] * n
            st_es = [None] * n
            st_attT = [None] * n
            st_ctx = [None] * n

            def stage_a(i):
                b, ht = units[i]
                tsl = slice(b * 128, (b + 1) * 128)
                s_ps = sc.tile([128, 512], F32, tag="sc", name="s_ps")
                attq = sxp.tile([128, HS], BF16, tag="attq", name="attq")
                es = rwp.tile([128, 2], F32, tag="es", name="es")
                for sub in range(2):
                    hp = sub * 64
                    csl = slice(sub * 128, (sub + 1) * 128)
                    nc.tensor.matmul(s_ps[:, csl],
                                     QT[ht][hp:hp + 64, tsl],
                                     KT[ht][hp:hp + 64, tsl],
                                     start=True, stop=True)
                    nc.scalar.activation(attq[:, csl], s_ps[:, csl], AF.Exp,
                                         scale=INV_SQRT_D,
                                         accum_out=es[:, sub:sub + 1])
                st_score[i] = s_ps
                st_attq[i] = attq
                st_es[i] = es

            def stage_b(i):
                rec = rwp.tile([128, 2], F32, tag="rec", name="rec")
                nc.vector.reciprocal(rec[:], st_es[i][:])
                atn = sxp.tile([128, HS], BF16, tag="atn", name="atn")
                nc.vector.tensor_scalar_mul(atn[:, 0:128],
                                            st_attq[i][:, 0:128],
                                            rec[:, 0:1])
                nc.vector.tensor_scalar_mul(atn[:, 128:256],
                                            st_attq[i][:, 128:256],
                                            rec[:, 1:2])
                s_ps = st_score[i]
                nc.tensor.transpose(s_ps[:, 256:320].bitcast(BF16),
                                    atn[:, 0:128], ident[:])
                nc.tensor.transpose(s_ps[:, 320:384].bitcast(BF16),
                                    atn[:, 128:256], ident[:])
                attT = sxp.tile([128, HS], BF16, tag="attT", name="attT")
                nc.vector.tensor_copy(attT[:], s_ps[:, 256:384].bitcast(BF16))
                st_attT[i] = attT

            def stage_e(i):
                b, ht = units[i]
                attT = st_attT[i]
                c_ps = cp.tile([128, 128], F32, tag="cp", name="c_ps")
                nc.tensor.matmul(c_ps[0:64, :],
                                 Vt[b][:, ht * 128:ht * 128 + 64],
                                 attT[:, 0:128], start=True, stop=True,
                                 tile_position=(0, 0))
                nc.tensor.matmul(c_ps[64:128, :],
                                 Vt[b][:, ht * 128 + 64:ht * 128 + 128],
                                 attT[:, 128:256], start=True, stop=True,
                                 tile_position=(0, 64))
                st_ctx[i] = c_ps

            def stage_f(i):
                b, ht = units[i]
                tsl = slice(b * 128, (b + 1) * 128)
                nc.vector.tensor_copy(CT[ht][:, tsl], st_ctx[i][:])
                st_score[i] = st_attq[i] = st_es[i] = None
                st_attT[i] = st_ctx[i] = None

            if _V2_SKIP_ATTN:
                for b, ht in units:
                    nc.vector.memset(CT[ht][:, b * 128:(b + 1) * 128], 0.01)
                for f in fillers:
                    f()
                return
            for i in range(n + 3):
                if 0 <= i - 3:
                    stage_f(i - 3)
                if i < n:
                    stage_a(i)
                if fillers:
                    fillers.pop(0)()
                if 0 <= i - 1 < n:
                    stage_b(i - 1)
                if 0 <= i - 2 < n:
                    stage_e(i - 2)
            for f in fillers:
                f()
            del st_score, st_attq, st_es, st_attT, st_ctx

        # ---- layernorm over one token half ----
        def ln_copy_out(h, final_out):
            hsl = slice(h * HS, (h + 1) * HS)
            for mt in range(H_TILES):
                if final_out is None:
                    nc.vector.tensor_copy(Xn[mt][:, hsl], X[mt][:, hsl])
                else:
                    xo = tmp.tile([128, HS], F32, tag="xo", name="xo")
                    nc.vector.tensor_copy(xo[:], X[mt][:, hsl])
                    nc.sync.dma_start(
                        final_out[mt][:, h * HS:(h + 1) * HS], xo[:])

        def ln_stats(rows, h):
            """S2 colsum chain + per-token stat rows (vector/scalar only —
            no PE work that could stall the engine FIFO behind it)."""
            for mt in range(H_TILES):
                nc.tensor.matmul(rows[0:1, 256:512], ones_cb[:],
                                 X2b[mt][:], start=(mt == 0),
                                 stop=(mt == H_TILES - 1))
            mu = rwp.tile([1, HS], F32, tag="mu", name="mu")
            nc.vector.tensor_scalar_mul(mu[:], rows[0:1, 0:256], INV_H)
            mu2 = rwp.tile([1, HS], F32, tag="mu2", name="mu2")
            nc.vector.tensor_tensor(out=mu2[:], in0=mu[:], in1=mu[:],
                                    op=OP.mult)
            var = rwp.tile([1, HS], F32, tag="var", name="var")
            nc.vector.scalar_tensor_tensor(out=var[:], in0=rows[0:1, 256:512],
                                           scalar=INV_H, in1=mu2[:],
                                           op0=OP.mult, op1=OP.subtract)
            sd = rwp.tile([1, HS], F32, tag="sd", name="sd")
            nc.scalar.activation(sd[:], var[:], AF.Sqrt, bias=eps_c[0:1, :])
            rstd = rwp.tile([1, HS], F32, tag="rstd", name="rstd")
            nc.vector.reciprocal(rstd[:], sd[:])
            cneg = rwp.tile([1, HS], BF16, tag="cneg", name="cneg")
            nc.vector.scalar_tensor_tensor(out=cneg[:], in0=mu[:],
                                           scalar=-1.0, in1=rstd[:],
                                           op0=OP.mult, op1=OP.mult)
            return rstd, cneg

        def ln_finish(rows, stats, h, final_out=None):
            hsl = slice(h * HS, (h + 1) * HS)
            rstd, cneg = stats
            # broadcast rstd/cneg into the rows bank (stats already consumed)
            nc.tensor.matmul(rows[:, 0:256], ones_rf[:], rstd[0:1, :],
                             start=True, stop=True)
            nc.tensor.matmul(rows[:, 256:512], ones_rb[:], cneg[0:1, :],
                             start=True, stop=True)
            for mt in range(H_TILES):
                t = tmp.tile([128, HS], F32, tag="tmp", name="t")
                nc.vector.tensor_tensor(out=t[:], in0=X[mt][:, hsl],
                                        in1=rows[:, 0:256], op=OP.mult)
                if final_out is None:
                    nc.vector.tensor_tensor(out=X[mt][:, hsl], in0=t[:],
                                            in1=rows[:, 256:512], op=OP.add)
                    nc.vector.tensor_copy(Xn[mt][:, hsl], X[mt][:, hsl])
                else:
                    xo = tmp.tile([128, HS], F32, tag="xo", name="xo")
                    nc.vector.tensor_tensor(out=xo[:], in0=t[:],
                                            in1=rows[:, 256:512], op=OP.add)
                    nc.sync.dma_start(final_out[mt][:, h * HS:(h + 1) * HS],
                                      xo[:])

        def ln_half(rows, h, final_out=None):
            if _V2_SKIP_LN:
                return ln_copy_out(h, final_out)
            stats = ln_stats(rows, h)
            ln_finish(rows, stats, h, final_out)

        def out_chain_ln(wsb, rhs_tiles, rows, mt, h, n_k=H_TILES, kofs=0):
            """projection chain for out-tile mt over token half h, then
            residual-add into X and the LN producer ops (colsum + square)."""
            hsl = slice(h * HS, (h + 1) * HS)
            ps = mm.tile([128, HS], F32, tag="mm", name="ops")
            for k in range(n_k):
                kc, kk = divmod(k + kofs, H_TILES)
                nc.tensor.matmul(
                    ps[:],
                    wsb[kc][:, kk * 768 + mt * 128:kk * 768 + mt * 128 + 128],
                    rhs_tiles[k][:, hsl], start=(k == 0), stop=(k == n_k - 1))
            nc.vector.scalar_tensor_tensor(out=X[mt][:, hsl], in0=ps[:],
                                           scalar=0.0, in1=X[mt][:, hsl],
                                           op0=OP.add, op1=OP.add)
            xb = tmp.tile([128, HS], BF16, tag="xb", name="xb")
            nc.vector.tensor_copy(xb[:], X[mt][:, hsl])
            nc.tensor.matmul(rows[0:1, 0:256], ones_cb[:], xb[:],
                             start=(mt == 0), stop=(mt == H_TILES - 1))
            nc.scalar.square(X2b[mt][:], X[mt][:, hsl])

        def ffn1_half(w1c, h):
            hsl = slice(h * HS, (h + 1) * HS)
            for fc in range(4):
                for fm in range(H_TILES):
                    g = fc * 6 + fm
                    ps = mm.tile([128, HS], F32, tag="mm", name="gps")
                    for kt in range(H_TILES):
                        nc.tensor.matmul(
                            ps[:],
                            w1c[fc][:, kt * 768 + fm * 128:
                                    kt * 768 + fm * 128 + 128],
                            Xn[kt][:, hsl], start=(kt == 0),
                            stop=(kt == H_TILES - 1))
                    nc.scalar.activation(G[g][:, hsl], ps[:],
                                         AF.Gelu_apprx_tanh)

        pending = [None]  # LN2(h1) finish of the previous layer
        for layer in range(n_layers):
            base = layer * 12
            wq = wp.tile([128, 4608], BF16, tag="w", name="wq")
            nc.sync.dma_start(wq[:], wblk[base + 0])
            wk = wp.tile([128, 4608], BF16, tag="w", name="wk")
            nc.sync.dma_start(wk[:], wblk[base + 1])
            wv = wp.tile([128, 4608], BF16, tag="w", name="wv")
            nc.sync.dma_start(wv[:], wblk[base + 2])
            wo = wp.tile([128, 4608], BF16, tag="w", name="wo")
            nc.sync.dma_start(wo[:], wblk[base + 3])

            # Q/K/V over half 0 (ready first), then flush the previous
            # layer's LN2(h1) finish, then attention b0/b1 with the half-1
            # projections as PE filler work; attention b2/b3 gets the
            # O-projection of half 0 as filler.
            qk_chain(wq, QT, 0)
            qk_chain(wk, KT, 0)
            v_chains(wv, 0)
            v_chains(wv, 1)
            if pending[0] is not None:
                pending[0]()
                pending[0] = None
            fill1 = (
                [lambda mt=mt: qk_chain_one(wq, QT, 1, mt)
                 for mt in range(H_TILES)]
                + [lambda mt=mt: qk_chain_one(wk, KT, 1, mt)
                   for mt in range(H_TILES)]
                + [lambda: v_chains(wv, 2), lambda: v_chains(wv, 3)])
            attn_units([(b, ht) for b in (0, 1) for ht in range(H_TILES)],
                       fill1)
            rows_o0 = rws.tile([128, 512], F32, tag="rows", name="rows")
            fill2 = [lambda mt=mt: out_chain_ln([wo], CT, rows_o0, mt, 0)
                     for mt in range(H_TILES)]
            attn_units([(b, ht) for b in (2, 3) for ht in range(H_TILES)],
                       fill2)

            # ---- LN1(h0) stats | O(h1) | LN1(h0) fin | LN1(h1) stats |
            #      FFN1(h0) | LN1(h1) fin | FFN1(h1) ----
            w1c = []
            for c in range(4):
                wt_ = wp.tile([128, 4608], BF16, tag="w", name=f"w1c{c}")
                nc.sync.dma_start(wt_[:], wblk[base + 4 + c])
                w1c.append(wt_)
            st_o0 = ln_stats(rows_o0, 0)
            rows_o1 = rws.tile([128, 512], F32, tag="rows", name="rows")
            for mt in range(H_TILES):
                out_chain_ln([wo], CT, rows_o1, mt, 1)
            ln_finish(rows_o0, st_o0, 0)
            st_o1 = ln_stats(rows_o1, 1)
            ffn1_half(w1c, 0)
            ln_finish(rows_o1, st_o1, 1)
            ffn1_half(w1c, 1)

            # ---- FFN2 + LN2, halves pipelined across the layer edge ----
            w2c = []
            for c in range(4):
                wt_ = wp.tile([128, 4608], BF16, tag="w", name=f"w2c{c}")
                nc.sync.dma_start(wt_[:], wblk[base + 8 + c])
                w2c.append(wt_)
            is_last = layer == n_layers - 1
            fo = [xout[i] for i in range(H_TILES)] if is_last else None
            rows_f0 = rws.tile([128, 512], F32, tag="rows", name="rows")
            for mt in range(H_TILES):
                out_chain_ln(w2c, G, rows_f0, mt, 0, n_k=24)
            st_f0 = ln_stats(rows_f0, 0)
            rows_f1 = rws.tile([128, 512], F32, tag="rows", name="rows")
            for mt in range(H_TILES):
                out_chain_ln(w2c, G, rows_f1, mt, 1, n_k=24)
            ln_finish(rows_f0, st_f0, 0, final_out=fo)
            st_f1 = ln_stats(rows_f1, 1)
            pending[0] = (lambda r=rows_f1, s=st_f1, f=fo:
                          ln_finish(r, s, 1, final_out=f))
        pending[0]()
    return nc


# ---------------------------------------------------------------------------
# Host-side prep / finish
# ---------------------------------------------------------------------------
def _pack768(w):
    return np.ascontiguousarray(
        w.reshape(6, 128, 768).transpose(1, 0, 2).reshape(128, 4608)
    ).astype(ml_dtypes.bfloat16)


def _host_ln(x, s, b, eps=EPS):
    mu = x.mean(-1, keepdims=True)
    var = ((x - mu) ** 2).mean(-1, keepdims=True)
    return s * (x - mu) / np.sqrt(var + eps) + b


def _prep_x0(inputs):
    idx = np.asarray(inputs["fol_bert_indices"]).astype(np.int64)
    typ = np.asarray(inputs["fol_bert_type"]).astype(np.int64)
    we = np.asarray(inputs["word_emb"], dtype=np.float32)
    emb = (we[idx].astype(np.float64)
           + np.asarray(inputs["pos_emb"], dtype=np.float64)[None]
           + np.asarray(inputs["type_emb"], dtype=np.float64)[typ])
    x0 = _host_ln(emb, np.asarray(inputs["emb_ln_s"], dtype=np.float64),
                  np.asarray(inputs["emb_ln_b"], dtype=np.float64))
    return x0.astype(np.float32)


def _pack_weights(inputs, n_layers=12):
    Wq = np.asarray(inputs["Wq"], dtype=np.float32)
    Wk = np.asarray(inputs["Wk"], dtype=np.float32)
    Wv = np.asarray(inputs["Wv"], dtype=np.float32)
    Wo = np.asarray(inputs["Wo"], dtype=np.float32)
    W1 = np.asarray(inputs["W1"], dtype=np.float32)
    W2 = np.asarray(inputs["W2"], dtype=np.float32)
    blocks = []
    for l in range(n_layers):
        blocks += [_pack768(Wq[l]), _pack768(Wk[l]), _pack768(Wv[l]),
                   _pack768(Wo[l])]
        blocks += [_pack768(W1[l][:, c * 768:(c + 1) * 768]) for c in range(4)]
        blocks += [_pack768(W2[l][c * 768:(c + 1) * 768, :]) for c in range(4)]
    return np.stack(blocks)


def _pack_bias(inputs, n_layers=12):
    pb = np.zeros((n_layers, 128, 48), np.float32)
    for l in range(n_layers):
        pb[l, :, 0:6] = np.asarray(inputs["bq"])[l].reshape(6, 128).T
        pb[l, :, 6:12] = np.asarray(inputs["bk"])[l].reshape(6, 128).T
        pb[l, :, 12:18] = np.asarray(inputs["bo"])[l].reshape(6, 128).T
        pb[l, :, 18:24] = np.asarray(inputs["b2"])[l].reshape(6, 128).T
        pb[l, :, 24:48] = np.asarray(inputs["b1"])[l].reshape(24, 128).T
    bv = np.ascontiguousarray(
        np.asarray(inputs["bv"], dtype=np.float32).reshape(n_layers, 1, 768))
    return pb, bv


def _pack_ln(inputs, n_layers=12):
    ln = np.zeros((n_layers, 128, 24), np.float32)
    for l in range(n_layers):
        ln[l, :, 0:6] = np.asarray(inputs["ln1_s"])[l].reshape(6, 128).T
        ln[l, :, 6:12] = np.asarray(inputs["ln1_b"])[l].reshape(6, 128).T
        ln[l, :, 12:18] = np.asarray(inputs["ln2_s"])[l].reshape(6, 128).T
        ln[l, :, 18:24] = np.asarray(inputs["ln2_b"])[l].reshape(6, 128).T
    return ln


def _bias_flags(inputs):
    use_bias = any(
        np.abs(np.asarray(inputs[k])).max() > 0
        for k in ("bq", "bk", "bv", "bo", "b1", "b2"))
    use_ln = (np.abs(np.asarray(inputs["ln1_s"]) - 1).max() > 0
              or np.abs(np.asarray(inputs["ln2_s"]) - 1).max() > 0
              or np.abs(np.asarray(inputs["ln1_b"])).max() > 0
              or np.abs(np.asarray(inputs["ln2_b"])).max() > 0)
    return bool(use_bias), bool(use_ln)


_BUILD_CACHE = {}


def _get_module(use_bias, use_ln_affine):
    key = (use_bias, use_ln_affine)
    if key not in _BUILD_CACHE:
        if not use_bias and not use_ln_affine:
            _BUILD_CACHE[key] = build_bert_v2(12)
        else:
            _BUILD_CACHE[key] = build_bert(12, use_bias, use_ln_affine)
    return _BUILD_CACHE[key]


def run_device(inputs, trace=False):
    """Run the 12-layer device kernel; returns (x12 [32,128,768] f32, results)."""
    from concourse import bass_utils
    use_bias, use_ln = _bias_flags(inputs)
    mask = np.asarray(inputs["fol_bert_mask"])
    if not np.all(mask == 1):
        raise NotImplementedError(
            "kernel specialized for the all-ones attention mask that "
            "setup_inputs() produces")
    nc = _get_module(use_bias, use_ln)
    x0 = _prep_x0(inputs)
    wblk = _pack_weights(inputs)
    extra = {}
    if use_bias:
        pb, bv = _pack_bias(inputs)
        extra["pbias"], extra["bvrow"] = pb, bv
    if use_ln:
        extra["lnsb"] = _pack_ln(inputs)
    in_maps = []
    for c in range(N_CORES):
        xt = np.ascontiguousarray(
            x0[c * B_LOC:(c + 1) * B_LOC].reshape(T, H).T).reshape(6, 128, T)
        in_maps.append({"x0t": xt, "wblk": wblk, **extra})
    if trace:
        os.environ.pop("BASS_NEVER_TRACE", None)
    res = bass_utils.run_bass_kernel_spmd(
        nc, in_maps, core_ids=list(range(N_CORES)), trace=trace)
    parts = []
    for c in range(N_CORES):
        xt = res.results[c]["xout"].reshape(H, T).T
        parts.append(xt.reshape(B_LOC, S, H))
    return np.concatenate(parts, 0), res


def kernel(**inputs) -> np.ndarray:
    x12, _ = run_device(inputs, trace=False)
    idx = np.asarray(inputs["fol_bert_indices"]).astype(np.int64)
    valid = (idx != 0).astype(np.float64)[..., None]
    x = x12.astype(np.float64)
    pooled = (x * valid).sum(1) / np.maximum(valid.sum(1), 1.0)
    out = pooled @ np.asarray(inputs["d2_W"], dtype=np.float64) \
        + np.asarray(inputs["d2_b"], dtype=np.float64)
    lab = np.asarray(inputs["word_emb"], dtype=np.float64)[
        np.asarray(inputs["prompt_label_idx"]).astype(np.int64)[0]]
    return (out @ lab.T).astype(np.float32)

